# revision 16
# baseline (speedup 1.0000x reference)
"""Trainium2 Bass kernel for nn_AttentionBlock_14267881357776.

Full attention block: q/k get positional encodings, three 256-wide input
projections, scaled dot-product attention with softmax over Tk, output
projection.  B=16, Tq=Tk=2048, C=A=E=256, fp32.

Sharding: pure data parallel over batch — each of the 8 NeuronCores
processes 2 full batch elements.  No collectives.

Per-core dataflow (per batch element):
  1. DMA q,k,v [2048,256] natural layout into SBUF.
  2. PE-transpose q,k,v into [C, T] layout (fp32, identity matmul);
     DVE evictions add the (host-precomputed, transposed) positional
     encoding for q/k and round v to float32r.
  3. Projections: QpT/KpT = W.T-blocks @ xT in exact fp32 (accuracy of the
     scores path matters: scores sigma~24 makes softmax near-one-hot, so
     argmax flips from TF32-class rounding would blow the error budget);
     Vp in float32r (post-softmax accuracy class).
  4. Per 128-row q tile: S = QpT.T @ KpT in fp32 (4 psum banks), row max
     (DVE, negated), exp via ACT with fused accumulation (softmax sum),
     P written as float32r.
  5. PE-transpose P blocks (f32r, 1.5 cyc/col), ctx = PT.T @ Vp (f32r),
     scale rows by 1/sum (rsqrt(Tk) is folded into Wo on the host),
     PE-transpose ctx, out = ctxT.T @ Wo_r + bo, DMA out.
"""

import os

import numpy as np

import concourse.bass as bass
import concourse.mybir as mybir
import concourse.tile as tile
from concourse import bacc
from concourse.bass_utils import run_bass_kernel_spmd
from concourse.masks import make_identity

F32 = mybir.dt.float32
F32R = mybir.dt.float32r
BF16 = mybir.dt.bfloat16
PV_DT = F32R  # dtype of the softmax/value pipeline (P, Vp); BF16 is ~15us faster but 13x less accurate
AF = mybir.ActivationFunctionType
AX = mybir.AxisListType

B, TQ, TK = 16, 2048, 2048
C, A, E = 256, 256, 256
NCORES = 8
BPC = B // NCORES  # batches per core
NQT = TQ // 128  # q tiles per batch
NKT = TK // 128  # k tiles per batch

_NC_CACHE = {}
LAST_RESULT = None  # BassKernelResults of the most recent run (for test.py)


def _positional_enc_host() -> np.ndarray:
    """[C, T] transposed positional encoding, computed with jax on CPU so the
    sin/cos of large angles match the fp32 reference bit-for-bit."""
    import jax
    import jax.numpy as jnp

    def f():
        pos = jnp.arange(TQ, dtype=jnp.float32)[:, None]
        kk = jnp.arange(C)
        power = jnp.power(10000.0, (2 * (kk // 2)).astype(jnp.float32) / C)
        ang = (pos * pos) * 1.0 / power
        pe = jnp.zeros((TQ, C), dtype=jnp.float32)
        pe = pe.at[:, 0::2].set(jnp.sin(ang[:, 0::2]))
        pe = pe.at[:, 1::2].set(jnp.cos(ang[:, 1::2]))
        return pe

    with jax.default_device(jax.devices("cpu")[0]):
        pe = np.asarray(jax.jit(f)())
    return np.ascontiguousarray(pe.T)


def _build(bench_reps=0):
    if bench_reps in _NC_CACHE:
        return _NC_CACHE[bench_reps]

    nc = bacc.Bacc()
    q_in = nc.declare_dram_parameter("q", [BPC, TQ, C], F32, isOutput=False)
    k_in = nc.declare_dram_parameter("k", [BPC, TK, C], F32, isOutput=False)
    v_in = nc.declare_dram_parameter("v", [BPC, TK, C], F32, isOutput=False)
    w1_in = nc.declare_dram_parameter("w1", [C, A], F32, isOutput=False)
    w2_in = nc.declare_dram_parameter("w2", [C, A], F32, isOutput=False)
    w3_in = nc.declare_dram_parameter("w3", [C, A], F32, isOutput=False)
    wo_in = nc.declare_dram_parameter("wo", [A, E], F32, isOutput=False)
    b1_in = nc.declare_dram_parameter("b1", [A], F32, isOutput=False)
    b2_in = nc.declare_dram_parameter("b2", [A], F32, isOutput=False)
    b3_in = nc.declare_dram_parameter("b3", [A], F32, isOutput=False)
    bo_in = nc.declare_dram_parameter("bo", [E], F32, isOutput=False)
    pe_in = nc.declare_dram_parameter("pe_t", [C, TQ], F32, isOutput=False)
    out_ext = nc.declare_dram_parameter("out", [BPC, TQ, E], F32, isOutput=True)

    with tile.TileContext(nc) as tc:
        with (
            tc.tile_pool(name="consts", bufs=1) as consts,
            tc.tile_pool(name="raw", bufs=2) as raw,
            tc.tile_pool(name="inT", bufs=2) as inT,
            tc.tile_pool(name="proj", bufs=1) as proj,
            tc.tile_pool(name="pp", bufs=2) as ppool,
            tc.tile_pool(name="ptp", bufs=2) as ptpool,
            tc.tile_pool(name="osb", bufs=2) as osb,
            tc.tile_pool(name="small", bufs=4) as small,
            tc.tile_pool(name="s_psum", bufs=4, space="PSUM") as s_psum,
            tc.tile_pool(name="tp_psum", bufs=2, space="PSUM") as tp_psum,
            tc.tile_pool(name="m_psum", bufs=2, space="PSUM") as m_psum,
        ):
            # ---- constants ----
            ident = consts.tile([128, 128], F32)
            make_identity(nc, ident[:])
            ident_r = consts.tile([128, 128], F32R)
            nc.vector.tensor_copy(ident_r[:], ident[:])
            ident_pv = consts.tile([128, 128], PV_DT)
            nc.vector.tensor_copy(ident_pv[:], ident[:])

            # Prefetch batch 0's q chunks ahead of the (larger) constant
            # DMAs so the first PE transposes start as early as possible.
            pre_q = []
            for h in range(2 if not bench_reps else 0):
                t = raw.tile([128, 8, C], F32, tag="raw")
                nc.sync.dma_start(
                    t[:],
                    q_in[0, 1024 * h : 1024 * (h + 1)].rearrange(
                        "(tt p) c -> p tt c", p=128
                    ),
                )
                pre_q.append(t)

            # pe in 512-col pieces right behind the q prefetch, so the first
            # transpose evictions (which add pe) are not stuck behind one
            # monolithic 2MB DMA.
            pe_sb = consts.tile([128, 2, TQ], F32)
            pe_re = pe_in.rearrange("(t p) n -> p t n", p=128)
            for j in range(4):
                nc.sync.dma_start(
                    pe_sb[:, :, 512 * j : 512 * (j + 1)],
                    pe_re[:, :, 512 * j : 512 * (j + 1)],
                )

            w1_sb = consts.tile([128, 2, A], F32)
            nc.sync.dma_start(w1_sb[:], w1_in.rearrange("(t p) a -> p t a", p=128))
            w2_sb = consts.tile([128, 2, A], F32)
            nc.sync.dma_start(w2_sb[:], w2_in.rearrange("(t p) a -> p t a", p=128))
            w3_st = raw.tile([128, 2, A], F32, tag="raw")
            nc.sync.dma_start(w3_st[:], w3_in.rearrange("(t p) a -> p t a", p=128))
            w3_sb = consts.tile([128, 2, A], PV_DT)
            nc.vector.tensor_copy(w3_sb[:], w3_st[:])
            wo_st = raw.tile([128, 2, E], F32, tag="raw")
            nc.sync.dma_start(wo_st[:], wo_in.rearrange("(t p) e -> p t e", p=128))
            wo_sb = consts.tile([128, 2, E], F32R)
            nc.vector.tensor_copy(wo_sb[:], wo_st[:])

            b1_sb = consts.tile([128, 2], F32)
            nc.sync.dma_start(b1_sb[:], b1_in.rearrange("(t p) -> p t", p=128))
            b2_sb = consts.tile([128, 2], F32)
            nc.sync.dma_start(b2_sb[:], b2_in.rearrange("(t p) -> p t", p=128))
            def bcast128(ap):
                return bass.AP(
                    tensor=ap.tensor, offset=ap.offset, ap=[[0, 128]] + list(ap.ap)
                )

            b3_sb = consts.tile([128, A], F32)
            nc.sync.dma_start(b3_sb[:], bcast128(b3_in[:]))
            bo_sb = consts.tile([128, E], F32)
            nc.sync.dma_start(bo_sb[:], bcast128(bo_in[:]))

            import contextlib

            loop_cm = (
                tc.For_i(0, bench_reps, 1) if bench_reps else contextlib.nullcontext()
            )
            with loop_cm:
              for b in range(BPC):
                # ---- load + transpose + project the three inputs ----
                projT = {}  # name -> (tile, layout info)
                for name, src, wt, bias_pp, bias_bc in (
                    ("q", q_in, w2_sb, b2_sb, None),
                    ("k", k_in, w1_sb, b1_sb, None),
                    ("v", v_in, w3_sb, None, b3_sb),
                ):
                    is_v = name == "v"
                    xT = inT.tile([128, 2, TQ], PV_DT if is_v else F32, tag="inT")
                    for h in range(2):
                        if b == 0 and name == "q" and not bench_reps:
                            x_sb = pre_q[h]
                        else:
                            x_sb = raw.tile([128, 8, C], F32, tag="raw")
                            nc.sync.dma_start(
                                x_sb[:],
                                src[b, 1024 * h : 1024 * (h + 1)].rearrange(
                                    "(tt p) c -> p tt c", p=128
                                ),
                            )
                        for ct in range(2):
                            for jl in range(2):
                                j = 2 * h + jl
                                tp = tp_psum.tile([128, 512], F32, tag="tp")
                                for i in range(4):
                                    tt = 4 * jl + i
                                    nc.tensor.matmul(
                                        tp[:, 128 * i : 128 * (i + 1)],
                                        x_sb[:, tt, 128 * ct : 128 * (ct + 1)],
                                        ident[:],
                                        is_transpose=True,
                                        start=(i == 0),
                                        stop=(i == 3),
                                    )
                                dst = xT[:, ct, 512 * j : 512 * (j + 1)]
                                if is_v:
                                    nc.vector.tensor_copy(dst, tp[:])
                                else:
                                    nc.vector.tensor_add(
                                        dst,
                                        tp[:],
                                        pe_sb[:, ct, 512 * j : 512 * (j + 1)],
                                    )
                    if is_v:
                        # Vp [k, a] in f32r: lhsT = vT block, rhs = W3 block
                        vp = proj.tile([128, 16, A], PV_DT, tag="vp")
                        for kt in range(NKT):
                            mp = m_psum.tile([128, A], F32, tag="m")
                            for ct in range(2):
                                nc.tensor.matmul(
                                    mp[:],
                                    xT[:, ct, 128 * kt : 128 * (kt + 1)],
                                    w3_sb[:, ct, :],
                                    start=(ct == 0),
                                    stop=(ct == 1),
                                )
                            nc.vector.tensor_add(vp[:, kt, :], mp[:], b3_sb[:])
                        projT["v"] = vp
                    else:
                        # XpT [a, t], exact fp32 psum split into f32r hi/lo
                        # (hi = f32r(x), lo = f32r(x - hi)) so the scores
                        # matmul can run as 3 fast f32r passes while staying
                        # fp32-accurate.
                        xpt_r = proj.tile([128, 2, TQ], F32R, tag=f"{name}ptr")
                        xpt_l = proj.tile([128, 2, TQ], F32R, tag=f"{name}ptl")
                        for at in range(2):
                            for qs in range(4):
                                sp = s_psum.tile([128, 512], F32, tag="s")
                                for ct in range(2):
                                    nc.tensor.matmul(
                                        sp[:],
                                        wt[:, ct, 128 * at : 128 * (at + 1)],
                                        xT[:, ct, 512 * qs : 512 * (qs + 1)],
                                        start=(ct == 0),
                                        stop=(ct == 1),
                                    )
                                dst_r = xpt_r[:, at, 512 * qs : 512 * (qs + 1)]
                                nc.vector.tensor_scalar_add(
                                    dst_r,
                                    in0=sp[:],
                                    scalar1=bias_pp[:, at : at + 1],
                                )
                                nc.vector.scalar_tensor_tensor(
                                    out=xpt_l[:, at, 512 * qs : 512 * (qs + 1)],
                                    in0=sp[:],
                                    scalar=bias_pp[:, at : at + 1],
                                    in1=dst_r,
                                    op0=mybir.AluOpType.add,
                                    op1=mybir.AluOpType.subtract,
                                )
                        projT[name] = (xpt_r, xpt_l)

                (qpt_r, qpt_l), (kpt_r, kpt_l), vp = (
                    projT["q"],
                    projT["k"],
                    projT["v"],
                )

                # ---- attention per q tile ----
                for qt in range(NQT):
                    s_ps = []
                    for ks in range(4):
                        sp = s_psum.tile([128, 512], F32, tag="s")
                        idx = 0
                        for ql, kl in (
                            (qpt_r, kpt_r),
                            (qpt_l, kpt_r),
                            (qpt_r, kpt_l),
                        ):
                            for at in range(2):
                                nc.tensor.matmul(
                                    sp[:],
                                    ql[:, at, 128 * qt : 128 * (qt + 1)],
                                    kl[:, at, 512 * ks : 512 * (ks + 1)],
                                    start=(idx == 0),
                                    stop=(idx == 5),
                                )
                                idx += 1
                        s_ps.append(sp)

                    # Chunked softmax over the two Tk halves: each half uses
                    # its own max for exp (stable), and the halves are
                    # combined at ctx eviction with per-row rescale factors
                    # exp(max_h - max_total) / sum_total.  This lets the PT
                    # transposes and ctx matmuls of half 0 start while half
                    # 1's scores/stats are still in flight.
                    max4 = small.tile([128, 4], F32, tag="max4")
                    negmaxP = small.tile([128, 2], F32, tag="negmaxP")
                    sumP = small.tile([128, 2], F32, tag="sumP")
                    p_sb = ppool.tile([128, TK], PV_DT, tag="p")
                    pt_sb = ptpool.tile([128, 16, 128], PV_DT, tag="pt")
                    half_ctx = []
                    for hf in range(2):
                        for ksl in range(2):
                            ks = 2 * hf + ksl
                            nc.vector.reduce_max(
                                max4[:, ks : ks + 1], s_ps[ks][:], axis=AX.X
                            )
                        negmax_h = negmaxP[:, hf : hf + 1]
                        nc.vector.reduce_max(
                            negmax_h,
                            max4[:, 2 * hf : 2 * hf + 2],
                            axis=AX.X,
                            negate=True,
                        )
                        sum2 = small.tile([128, 2], F32, tag=f"sum2_{hf}")
                        for ksl in range(2):
                            ks = 2 * hf + ksl
                            nc.scalar.activation(
                                p_sb[:, 512 * ks : 512 * (ks + 1)],
                                s_ps[ks][:],
                                AF.Exp,
                                bias=negmax_h,
                                scale=1.0,
                                accum_out=sum2[:, ksl : ksl + 1],
                            )
                        nc.vector.reduce_sum(
                            sumP[:, hf : hf + 1], sum2[:], axis=AX.X
                        )
                        # transpose this half's 8 P blocks (4 per psum bank)
                        for jl in range(2):
                            j = 2 * hf + jl
                            tp = tp_psum.tile([128, 512], PV_DT, tag="tp")
                            for i in range(4):
                                kt = 4 * j + i
                                nc.tensor.matmul(
                                    tp[:, 128 * i : 128 * (i + 1)],
                                    p_sb[:, 128 * kt : 128 * (kt + 1)],
                                    ident_pv[:],
                                    is_transpose=True,
                                    start=(i == 0),
                                    stop=(i == 3),
                                )
                            nc.scalar.copy(
                                pt_sb[:, 4 * j : 4 * (j + 1), :].rearrange(
                                    "p a b -> p (a b)"
                                ),
                                tp[:],
                            )
                        # this half's ctx partial (8-step accumulation)
                        cph = m_psum.tile([128, A], F32, tag="m")
                        for ktl in range(8):
                            kt = 8 * hf + ktl
                            nc.tensor.matmul(
                                cph[:],
                                pt_sb[:, kt, :],
                                vp[:, kt, :],
                                start=(ktl == 0),
                                stop=(ktl == 7),
                            )
                        half_ctx.append(cph)

                    # combine halves: corr_h = exp(max_h - max_t), weights
                    # c_h = corr_h / (corr_0 sum_0 + corr_1 sum_1)
                    negmax_t = small.tile([128, 1], F32, tag="negmax_t")
                    nc.vector.reduce_max(
                        negmax_t[:], max4[:], axis=AX.X, negate=True
                    )
                    corrP = small.tile([128, 2], F32, tag="corrP")
                    nc.scalar.activation(
                        corrP[:], negmaxP[:], AF.Exp, bias=negmax_t[:], scale=-1.0
                    )
                    prod = small.tile([128, 2], F32, tag="prod")
                    nc.vector.tensor_mul(prod[:], corrP[:], sumP[:])
                    sum_t = small.tile([128, 1], F32, tag="sum_t")
                    nc.vector.reduce_sum(sum_t[:], prod[:], axis=AX.X)
                    recip = small.tile([128, 1], F32, tag="recip")
                    nc.vector.reciprocal(recip[:], sum_t[:])
                    cP = small.tile([128, 2], F32, tag="cP")
                    nc.vector.tensor_scalar_mul(cP[:], in0=corrP[:], scalar1=recip[:])

                    ctmp = osb.tile([128, A], F32, tag="ctmp")
                    nc.vector.tensor_scalar_mul(
                        ctmp[:], in0=half_ctx[0][:], scalar1=cP[:, 0:1]
                    )
                    ctx_r = osb.tile([128, A], F32R, tag="ctxr")
                    nc.vector.scalar_tensor_tensor(
                        out=ctx_r[:],
                        in0=half_ctx[1][:],
                        scalar=cP[:, 1:2],
                        in1=ctmp[:],
                        op0=mybir.AluOpType.mult,
                        op1=mybir.AluOpType.add,
                    )

                    # transpose ctx (2 blocks into one psum bank)
                    ctp = m_psum.tile([128, A], F32R, tag="m")
                    for at in range(2):
                        nc.tensor.matmul(
                            ctp[:, 128 * at : 128 * (at + 1)],
                            ctx_r[:, 128 * at : 128 * (at + 1)],
                            ident_r[:],
                            is_transpose=True,
                            start=(at == 0),
                            stop=(at == 1),
                        )
                    ctxT = osb.tile([128, 2, 128], F32R, tag="ctxT")
                    nc.vector.tensor_copy(
                        ctxT[:].rearrange("p a b -> p (a b)"), ctp[:]
                    )

                    # out = ctxT.T @ Wo_r + bo
                    op = m_psum.tile([128, E], F32, tag="m")
                    for at in range(2):
                        nc.tensor.matmul(
                            op[:],
                            ctxT[:, at, :],
                            wo_sb[:, at, :],
                            start=(at == 0),
                            stop=(at == 1),
                        )
                    o_sb = osb.tile([128, E], F32, tag="o")
                    nc.vector.tensor_add(o_sb[:], op[:], bo_sb[:])
                    nc.sync.dma_start(
                        out_ext[b, 128 * qt : 128 * (qt + 1), :], o_sb[:]
                    )

    nc.compile()
    _NC_CACHE[bench_reps] = nc
    return nc


def kernel(
    query: np.ndarray,
    keys: np.ndarray,
    values: np.ndarray,
    W1: np.ndarray,
    b1: np.ndarray,
    W2: np.ndarray,
    b2: np.ndarray,
    W3: np.ndarray,
    b3: np.ndarray,
    Wo: np.ndarray,
    bo: np.ndarray,
) -> np.ndarray:
    global LAST_RESULT
    nc = _build()
    pe_t = _positional_enc_host()
    wo_scaled = np.ascontiguousarray(
        (Wo.astype(np.float32) * np.float32(1.0 / np.sqrt(TK))).astype(np.float32)
    )
    shared = {
        "w1": np.ascontiguousarray(W1, dtype=np.float32),
        "w2": np.ascontiguousarray(W2, dtype=np.float32),
        "w3": np.ascontiguousarray(W3, dtype=np.float32),
        "wo": wo_scaled,
        "b1": np.ascontiguousarray(b1, dtype=np.float32),
        "b2": np.ascontiguousarray(b2, dtype=np.float32),
        "b3": np.ascontiguousarray(b3, dtype=np.float32),
        "bo": np.ascontiguousarray(bo, dtype=np.float32),
        "pe_t": pe_t,
    }
    in_maps = []
    for i in range(NCORES):
        sl = slice(i * BPC, (i + 1) * BPC)
        in_maps.append(
            {
                "q": np.ascontiguousarray(query[sl], dtype=np.float32),
                "k": np.ascontiguousarray(keys[sl], dtype=np.float32),
                "v": np.ascontiguousarray(values[sl], dtype=np.float32),
                **shared,
            }
        )
    res = run_bass_kernel_spmd(
        nc,
        in_maps,
        list(range(NCORES)),
        trace=bool(os.environ.get("KERNEL_TRACE")),
    )
    LAST_RESULT = res
    out = np.empty((B, TQ, E), dtype=np.float32)
    for i in range(NCORES):
        out[i * BPC : (i + 1) * BPC] = res.results[i]["out"]
    return out
